# revision 4
# baseline (speedup 1.0000x reference)
"""CIN (Compressed Interaction Network) forward kernel for Trainium2.

Problem: x[B=1024, M=39, D=64] f32; W0[M, M, H1=128]; W1[M, H1, H2=128].
  h1 = einsum('bid,bjd,ijh->bhd', x, x, W0)
  h2 = einsum('bid,bjd,ijh->bhd', x, h1, W1)
  out = concat([h1, h2], axis=1).sum(-1)   -> [B, 256]

Strategy (data-parallel over B across 8 cores, 128 batches/core):
  Per (b, d) pair the einsum is a flattened outer product z[(i,j)] followed by
  a [K x 128] matmul (K1=1521, K2=4992). Per d-slice (128 b's on partitions):
    1. DVE builds Z[b, (i,j)] = x[b,i,d]*x[b,j,d] in one tensor_tensor op
       using step-0 (broadcast) access-pattern dims.
    2. PE transposes 128-col chunks of Z into PSUM (f32r), ACT copies to SBUF.
    3. f32r matmuls W_chunk.T @ Zt accumulate H^T[h, (d,b)] in PSUM (N=512,
       grouping 4 d-slices per matmul).
  Layer-1 output is de-transposed per d to feed the layer-2 Z build; layer-2
  PSUM accumulates across the entire kernel and is reduced at the end.
"""
import numpy as np

B, M, D = 1024, 39, 64
H1, H2 = 128, 128
NCORES = 8
BS = B // NCORES          # 128 batches per core
K1 = M * M                # 1521
NC1 = 12                  # ceil(K1/128); last chunk K=113
K2 = M * H1               # 4992
NC2 = K2 // 128           # 39
GD = 4                    # d-slices per matmul group (N = GD*128 = 512)
NG = D // GD              # 16 groups
LT = 3                    # layer-2 build split (i-ranges) per d
N = GD * 128              # 512


def _split_waits(nc, maxw=1):
    """This walrus build allows only one sem-wait per instruction; split
    Tile's multi-wait instructions into preceding single-wait NoOps."""
    import concourse.mybir as mybir

    n_new = 0
    for fn in nc.m.functions:
        for bb in fn.blocks:
            insts = bb.instructions
            out = []
            changed = False
            for inst in insts:
                si = inst.sync_info
                if si and si.on_wait and len(si.on_wait) > maxw:
                    waits = list(si.on_wait)
                    chunks = [waits[i:i + maxw] for i in range(0, len(waits), maxw)]
                    for ch in chunks[:-1]:
                        nop = mybir.InstNoOp(name=f"wsplit-{n_new}", ins=[], outs=[])
                        n_new += 1
                        nop.engine = inst.engine
                        nop.sync_info = mybir.SyncInfo(on_wait=ch, on_update=[])
                        out.append(nop)
                    inst.sync_info = mybir.SyncInfo(
                        on_wait=chunks[-1], on_update=list(si.on_update)
                    )
                    changed = True
                out.append(inst)
            if changed:
                bb.instructions = out
    return n_new


def _build_bass():
    import concourse.bass as bass
    import concourse.mybir as mybir
    import concourse.tile as tile
    from concourse import masks

    F32 = mybir.dt.float32
    F32R = mybir.dt.float32r
    MULT = mybir.AluOpType.mult

    nc = bass.Bass()
    x_d = nc.dram_tensor("x", [BS, M * D], F32, kind="ExternalInput")
    w0_d = nc.dram_tensor("w0", [K1, H1], F32R, kind="ExternalInput")
    w1_d = nc.dram_tensor("w1", [K2, H2], F32R, kind="ExternalInput")
    out_d = nc.dram_tensor("out", [BS, H1 + H2], F32, kind="ExternalOutput")

    with tile.TileContext(nc) as tc:
        with (
            tc.tile_pool(name="const", bufs=1) as const,
            tc.tile_pool(name="zp1", bufs=6) as zp1,
            tc.tile_pool(name="zp2", bufs=6) as zp2,
            tc.tile_pool(name="ztp", bufs=6) as ztp,
            tc.tile_pool(name="h1p", bufs=6) as h1pool,
            tc.tile_pool(name="ps_stage", bufs=2, space="PSUM") as ps_stage,
            tc.tile_pool(name="ps_h1", bufs=2, space="PSUM") as ps_h1,
            tc.tile_pool(name="ps_h2", bufs=1, space="PSUM") as ps_h2,
            tc.tile_pool(name="ps_det", bufs=2, space="PSUM") as ps_det,
        ):
            # ---- constants / inputs resident in SBUF ----
            ident32 = const.tile([128, 128], F32)
            masks.make_identity(nc, ident32[:])
            identr = const.tile([128, 128], F32R)
            nc.vector.tensor_copy(identr[:], ident32[:])

            x_sb = const.tile([BS, M * D], F32)
            nc.sync.dma_start(x_sb[:], x_d[:])
            w0_sb = const.tile([128, NC1 * H1], F32R)
            nc.sync.dma_start(
                w0_sb[:, :(NC1 - 1) * H1].rearrange("p (c h) -> p c h", c=NC1 - 1),
                w0_d[:(NC1 - 1) * 128].rearrange("(c p) h -> p c h", p=128),
            )
            nc.sync.dma_start(
                w0_sb[:K1 - (NC1 - 1) * 128, (NC1 - 1) * H1:],
                w0_d[(NC1 - 1) * 128:],
            )
            w1_sb = const.tile([128, NC2 * H2], F32R)
            nc.sync.dma_start(
                w1_sb[:].rearrange("p (c h) -> p c h", c=NC2),
                w1_d[:].rearrange("(c p) h -> p c h", p=128),
            )

            acc1 = const.tile([128, 128], F32)  # [b, h1] accumulator
            nc.gpsimd.memset(acc1[:], 0.0)

            # layer-2 PSUM accumulator, lives across the whole kernel
            h2ps = ps_h2.tile([128, N], F32)

            x3 = x_sb[:].rearrange("p (i d) -> p i d", i=M)  # [128, 39, 64]

            for g in range(NG):
                # ---------- layer 1: build Z1 for 4 d-slices ----------
                z1s = []
                for dd in range(GD):
                    d = g * GD + dd
                    xv = x3[:, :, d]  # [128, 39] stride-64 view
                    z1 = zp1.tile([128, K1], F32R)
                    nc.vector.tensor_tensor(
                        z1[:].rearrange("p (i j) -> p i j", i=M),
                        xv.unsqueeze(1).broadcast_to((128, M, M)),
                        xv.unsqueeze(2).broadcast_to((128, M, M)),
                        MULT,
                    )
                    z1s.append(z1)

                # ---------- layer 1: transpose + matmul ----------
                h1ps = ps_h1.tile([128, N], F32)
                for c in range(NC1):
                    kc = min(128, K1 - c * 128)
                    stage = ps_stage.tile([128, N], F32R)
                    for dd in range(GD):
                        nc.tensor.transpose(
                            stage[:kc, dd * 128:(dd + 1) * 128],
                            z1s[dd][:, c * 128:c * 128 + kc],
                            identr[:],
                        )
                    zt = ztp.tile([128, N], F32R)
                    nc.scalar.copy(zt[:kc], stage[:kc])
                    nc.tensor.matmul(
                        h1ps[:], w0_sb[:kc, c * H1:(c + 1) * H1], zt[:kc],
                        start=(c == 0), stop=(c == NC1 - 1),
                    )

                # ---------- extract H1 per d (de-transpose) + acc1 ----------
                h1ds = []
                for dd in range(GD):
                    h1t = h1pool.tile([128, 128], F32)
                    nc.scalar.copy(h1t[:], h1ps[:, dd * 128:(dd + 1) * 128])
                    det = ps_det.tile([128, 128], F32)
                    nc.tensor.transpose(det[:], h1t[:], ident32[:])
                    h1d = h1pool.tile([128, 128], F32)  # [b, j]
                    nc.scalar.copy(h1d[:], det[:])
                    h1ds.append(h1d)
                    nc.vector.tensor_tensor(acc1[:], acc1[:], h1d[:],
                                            mybir.AluOpType.add)

                # ---------- layer 2: build + transpose + matmul ----------
                for t in range(LT):
                    i0 = t * 13
                    ni = min(13, M - i0)
                    z2s = []
                    for dd in range(GD):
                        d = g * GD + dd
                        xv = x3[:, :, d]
                        z2 = zp2.tile([128, 13 * H1], F32R)
                        nc.vector.tensor_tensor(
                            z2[:, :ni * H1].rearrange("p (i j) -> p i j", i=ni),
                            h1ds[dd][:].unsqueeze(1).broadcast_to((128, ni, H1)),
                            xv[:, i0:i0 + ni].unsqueeze(2).broadcast_to(
                                (128, ni, H1)),
                            MULT,
                        )
                        z2s.append(z2)
                    for ci in range(ni):
                        c = i0 + ci
                        stage = ps_stage.tile([128, N], F32R)
                        for dd in range(GD):
                            nc.tensor.transpose(
                                stage[:, dd * 128:(dd + 1) * 128],
                                z2s[dd][:, ci * 128:(ci + 1) * 128],
                                identr[:],
                            )
                        zt = ztp.tile([128, N], F32R)
                        nc.scalar.copy(zt[:], stage[:])
                        nc.tensor.matmul(
                            h2ps[:], w1_sb[:, c * H2:(c + 1) * H2], zt[:],
                            start=(g == 0 and c == 0),
                            stop=(g == NG - 1 and c == NC2 - 1),
                        )

            # ---------- finalize ----------
            # h2ps[h, (dd, b)] accumulated over all groups; sum the 4 dd slots
            acc2h = const.tile([128, 128], F32)
            nc.scalar.copy(acc2h[:], h2ps[:, 0:128])
            for dd in range(1, GD):
                nc.vector.tensor_tensor(
                    acc2h[:], acc2h[:], h2ps[:, dd * 128:(dd + 1) * 128],
                    mybir.AluOpType.add,
                )
            det2 = ps_det.tile([128, 128], F32, tag="det")
            nc.tensor.transpose(det2[:], acc2h[:], ident32[:])
            acc2b = const.tile([128, 128], F32)
            nc.scalar.copy(acc2b[:], det2[:])

            nc.sync.dma_start(out_d[:, 0:H1], acc1[:])
            nc.sync.dma_start(out_d[:, H1:H1 + H2], acc2b[:])

    _split_waits(nc)
    return nc


_NC_CACHE = None


def _get_nc():
    global _NC_CACHE
    if _NC_CACHE is None:
        _NC_CACHE = _build_bass()
    return _NC_CACHE


def kernel(x, W0, W1, trace=False):
    from concourse.bass_utils import run_bass_kernel_spmd

    nc = _get_nc()

    w0f = np.ascontiguousarray(W0, dtype=np.float32).reshape(K1, H1)
    w1f = np.ascontiguousarray(W1, dtype=np.float32).reshape(K2, H2)
    xs = np.ascontiguousarray(x, dtype=np.float32).reshape(
        NCORES, BS, M * D)

    in_maps = [{"x": xs[i], "w0": w0f, "w1": w1f} for i in range(NCORES)]
    res = run_bass_kernel_spmd(nc, in_maps, list(range(NCORES)), trace=trace)
    out = np.concatenate([r["out"] for r in res.results], axis=0)
    if trace:
        return out, res
    return out
